# revision 34
# baseline (speedup 1.0000x reference)
"""Trainium2 Bass kernel for nn_HadaMard: fused proj + 2xLayerNorm + outer product.

Reference computation (per batch b, one NeuronCore per batch):
  qf = q[b].reshape(C1, N)           # [1024, 1024]
  proj = Wp @ qf (+ bp)              # [256, 1024]
  qn = LN_over_d(proj) * g1 + b1     # LN over the 256-channel dim
  xn = LN_over_e(x[b]) * g2 + b2     # LN over the 32-channel dim
  out[d*32+e, n] = qn[d, n] * xn[e, n]   # [8192, 1024]

Layout/strategy:
  - Output is transferred in bf16 (rel-err ~6e-3 << 2e-2 gate) and upcast on
    host: halves the dominant HBM write traffic.
  - Outer-product tiles are e-major: tile (md, e) holds out rows
    (128*md+p)*32+e for p in [0,128). The qn factor is the bf16 qn tile
    itself (no broadcast); the xn factor is one row broadcast to all 128
    partitions.
  - Row broadcasts go through a DRAM scratch roundtrip: A = xn is written
    once (ready early, x-side only), then each xbe tile is a stride-0
    partition-broadcast DMA read. These land on the DMA queues
    (sync/scalar/gpsimd) during the otherwise-idle window while the q-side
    LN chain runs, instead of loading the busy compute engines.
  - Elementwise products run on DVE (bf16 2x mode) and Pool, DMAs on
    sync/scalar/gpsimd, assigned by a static least-loaded balancer.
  - The q side is processed in two 512-column chunks (PSUM bank granularity);
    early e's run per-chunk products to start output DMA sooner, later e's
    run full-width products.
"""

import numpy as np

_CACHE = {}

B, C1, H, W = 8, 1024, 32, 32
C2 = 32
Cp = 256
N = H * W  # 1024
CD = Cp * C2  # 8192
EPS = 1e-5

CFG = {
    "esplit": 8,        # e < esplit: per-chunk products; else full-width
    "nwu": 2,           # PE warm-up matmuls
    "proj_order": "il", # "bb" = both proj chunks first; "il" = proj/ln interleaved
    "pool_mod": (3, 1), # Pool owns tiles with e % m == r
    "out_pat": ["sync", "scalar", "gpsimd", "sync", "scalar", "sync", "scalar"],
    "xbe_pat": ["gpsimd", "sync", "gpsimd", "scalar"],
    "q_eng": ["sync", "sync", "sync", "scalar", "scalar", "scalar", "gpsimd", "gpsimd"],
    "prefetch": 10,
    "head_out": ["sync"],
    "pf_eng": ["sync"],
    "tail_n": 0,
    "exit_eng": "scalar",
    "split_last": 0,
}


def _build_nc(simple):
    import concourse.bacc as bacc
    import concourse.bass as bass
    import concourse.mybir as mybir
    import concourse.tile as tile

    F32 = mybir.dt.float32
    F32R = mybir.dt.float32r
    BF16 = mybir.dt.bfloat16
    SQRT = mybir.ActivationFunctionType.Sqrt
    COPY = mybir.ActivationFunctionType.Copy
    MULT = mybir.AluOpType.mult
    ADD = mybir.AluOpType.add

    nc = bacc.Bacc(None, target_bir_lowering=False)

    q_d = nc.dram_tensor("qb", [C1, N], BF16, kind="ExternalInput")
    w_d = nc.dram_tensor("wT", [C1, Cp], BF16, kind="ExternalInput")
    x_d = nc.dram_tensor("x", [C2, N], F32, kind="ExternalInput")
    bp_d = nc.dram_tensor("bpc", [128, 2], F32, kind="ExternalInput")
    g1_d = nc.dram_tensor("g1c", [128, 2], F32, kind="ExternalInput")
    b1_d = nc.dram_tensor("b1c", [128, 2], F32, kind="ExternalInput")
    g2_d = nc.dram_tensor("g2r", [128, 1], F32, kind="ExternalInput")
    b2_d = nc.dram_tensor("b2r", [128, 1], F32, kind="ExternalInput")
    abuf_d = nc.dram_tensor("abuf", [128, 256], BF16, kind="Internal")  # packed: row 32a+e = A[e, 256a:]
    out_d = nc.dram_tensor("out", [CD, N], BF16, kind="ExternalOutput")

    # out view: row (md*128+p)*32+e  ->  [p, md, e, n]
    outv = out_d.rearrange("(md p e) n -> p md e n", md=2, p=128, e=C2)

    # ---- static least-loaded balancer (model-cost ns) ----
    clk = {"sync": 0.0, "scalar": 0.0, "gpsimd": 0.0, "vector": 0.0}

    def pick(cands, costs):
        e = min(cands, key=lambda x: clk[x])
        clk[e] += costs[e] if isinstance(costs, dict) else costs
        return e

    def charge(e, cost):
        clk[e] += cost

    DMA_ENGS = ["sync", "scalar", "gpsimd"]

    def dma_cost(bytes_per_part):
        return max(bytes_per_part * 0.3855, 500.0)

    def mul_costs(w):
        return {"vector": w * 1.0417 * 0.5 + 60.0, "gpsimd": w * 0.8333}

    CHUNKS = [(0, 512), (512, 1024)]

    with tile.TileContext(nc) as tc:
        with (
            tc.tile_pool(name="cst", bufs=1) as cst,
            tc.tile_pool(name="big", bufs=1) as big,
            tc.tile_pool(name="xbe", bufs=1) as xbp,
            tc.tile_pool(name="stg", bufs=7) as stg,
            tc.tile_pool(name="stc", bufs=4) as stc,
            tc.tile_pool(name="wrk", bufs=2) as wrk,
            tc.tile_pool(name="ps", bufs=5, space=bass.MemorySpace.PSUM) as ps,
            tc.tile_pool(name="wups", bufs=1, space=bass.MemorySpace.PSUM) as wups,
            tc.tile_pool(name="ps32", bufs=2, space=bass.MemorySpace.PSUM) as ps32,
        ):
            # ---- constants / warmup (t=0, no input deps) ----
            wu_l = cst.tile([128, 128], BF16, tag="wul")
            nc.vector.memset(wu_l[:], 0.5)
            wu_r = cst.tile([128, 256], BF16, tag="wur")
            nc.vector.memset(wu_r[:], 0.5)
            cq128 = cst.tile([128, 128], F32, tag="cq128")
            nc.vector.memset(cq128[:], 1.0 / Cp)
            cjx = cst.tile([128, 128], F32, tag="cjx")
            nc.vector.memset(cjx[:], 0.0)
            for a_ in range(4):
                nc.vector.memset(cjx[32 * a_ : 32 * (a_ + 1), 32 * a_ : 32 * (a_ + 1)], 1.0 / C2)
            eps32 = cst.tile([C2, 1], F32, tag="eps32")
            nc.vector.memset(eps32[:], EPS)
            eps128 = cst.tile([128, 1], F32, tag="eps128")
            nc.vector.memset(eps128[:], EPS)
            # preload the activation table early (ACT, off critical path)
            atl = cst.tile([C2, 1], F32, tag="atl")
            nc.scalar.copy(atl[:], eps32[:])
            nc.scalar.activation(atl[:], eps32[:], SQRT, bias=eps32[:])

            wu_ps = wups.tile([128, 256], F32, tag="wups")
            for i in range(CFG["nwu"]):
                nc.tensor.matmul(wu_ps[:], wu_l[:], wu_r[:], start=True, stop=True)

            def fillers(n):
                for _ in range(n):
                    nc.tensor.matmul(wu_ps[:], wu_l[:], wu_r[:], start=True, stop=True)

            # ---- input loads: q/w spread across all queues for fast proj
            # start; x packed (x4[32a+e, n'] = x[e, 256a+n']) on gpsimd ----
            w_sb = []
            for j in range(2):
                wt = big.tile([128, 4 * Cp], BF16, tag=f"w{j}")
                w_sb.append(wt)
                src = w_d[512 * j : 512 * (j + 1), :].rearrange("(c p) d -> p c d", c=4)
                dst = wt[:].rearrange("p (c d) -> p c d", c=4)
                eng = ["sync", "scalar"][j]
                charge(eng, dma_cost(2048))
                getattr(nc, eng).dma_start(dst, src)
            x_sb = cst.tile([128, 256], F32, tag="xs")
            charge("gpsimd", dma_cost(1024))
            nc.gpsimd.dma_start(
                x_sb[:], x_d.rearrange("e (a n) -> a e n", a=4)
            )
            q_sb = []
            q_eng = CFG["q_eng"]
            for k in range(8):
                qt = big.tile([128, N], BF16, tag=f"q{k}")
                q_sb.append(qt)
                charge(q_eng[k], dma_cost(2048))
                getattr(nc, q_eng[k]).dma_start(qt[:], q_d[128 * k : 128 * (k + 1), :])

            def wslice(k, md):
                j, c = divmod(k, 4)
                return w_sb[j][:, 256 * c + 128 * md : 256 * c + 128 * (md + 1)]

            def cload(dram, shape, tag):
                t = cst.tile(shape, F32, tag=tag)
                eng = pick(DMA_ENGS, dma_cost(shape[1] * 4))
                getattr(nc, eng).dma_start(t[:], dram[:])
                return t

            if not simple:
                bp_sb = cload(bp_d, [128, 2], "bp")
                g1_sb = cload(g1_d, [128, 2], "g1")
                b1_sb = cload(b1_d, [128, 2], "b1")
                g2_sb = cload(g2_d, [128, 1], "g2")
                b2_sb = cload(b2_d, [128, 1], "b2")

            # ---- x-side LN (packed [128,256]; independent of q) ----
            xsq = cst.tile([128, 256], F32, tag="xsq")
            nc.gpsimd.tensor_mul(xsq[:], x_sb[:], x_sb[:])
            charge("gpsimd", 256 * 0.8333)
            smx = ps32.tile([128, 256], F32, tag="s32")
            nc.tensor.matmul(
                smx[:], cjx[:].bitcast(F32R), x_sb[:].bitcast(F32R),
                start=True, stop=True,
            )
            sqx = ps32.tile([128, 256], F32, tag="s32")
            nc.tensor.matmul(
                sqx[:], cjx[:].bitcast(F32R), xsq[:].bitcast(F32R),
                start=True, stop=True,
            )
            mx = cst.tile([128, 256], F32, tag="mx")
            nc.scalar.copy(mx[:], smx[:])
            charge("scalar", 256 * 0.8333 + 370)
            mx2 = cst.tile([128, 256], F32, tag="mx2")
            nc.gpsimd.tensor_mul(mx2[:], mx[:], mx[:])
            charge("gpsimd", 256 * 0.8333)
            vx = cst.tile([128, 256], F32, tag="vx")
            nc.vector.tensor_sub(vx[:], sqx[:], mx2[:])
            charge("vector", 256 * 1.0417 + 125)
            sdx = cst.tile([128, 256], F32, tag="sdx")
            nc.scalar.activation(sdx[:], vx[:], SQRT, bias=eps128[:])
            charge("scalar", 256 * 0.8333 + 370)
            rsx = cst.tile([128, 256], F32, tag="rsx")
            nc.vector.reciprocal(rsx[:], sdx[:])
            charge("vector", 256 * 1.0417 + 60)
            xt = cst.tile([128, 256], F32, tag="xt")
            nc.gpsimd.tensor_sub(xt[:], x_sb[:], mx[:])
            charge("gpsimd", 256 * 0.8333)
            a_sb = cst.tile([128, 256], BF16, tag="a")
            if simple:
                nc.gpsimd.tensor_mul(a_sb[:], xt[:], rsx[:])
                charge("gpsimd", 256 * 0.8333)
            else:
                t3 = cst.tile([128, 256], F32, tag="t3")
                nc.gpsimd.tensor_mul(t3[:], xt[:], rsx[:])
                charge("gpsimd", 256 * 0.8333)
                nc.vector.tensor_scalar(
                    a_sb[:], t3[:], g2_sb[:, 0:1], b2_sb[:, 0:1], op0=MULT, op1=ADD
                )
                charge("vector", 256 * 1.0417 + 60)

            # A -> DRAM scratch (packed), then all 32 broadcast reads (fill
            # the DMA window while the q-side LN chain runs)
            charge("sync", dma_cost(512))
            nc.sync.dma_start(abuf_d[:], a_sb[:])
            abufv = abuf_d.rearrange("(a e) n -> e a n", a=4, e=C2)
            xbes = [None] * C2
            xbe_rr = [0]

            def emit_xbe(e, eng=None):
                if xbes[e] is not None:
                    return
                t = xbp.tile([128, N], BF16, tag=f"xbe{e}")
                src = abufv[e : e + 1, :, :].partition_broadcast(128)
                if eng is None:
                    pat = CFG["xbe_pat"]
                    eng = pat[xbe_rr[0] % len(pat)]
                    xbe_rr[0] += 1
                charge(eng, dma_cost(N * 2))
                getattr(nc, eng).dma_start(
                    t[:].rearrange("p (a n) -> p a n", a=4), src
                )
                xbes[e] = t

            pfe = CFG["pf_eng"]
            for e in range(CFG["prefetch"]):
                emit_xbe(e, pfe[e % len(pfe)])

            # ---- q-side: proj + LN -> cn (bf16, rstd folded in) ----
            cn = []
            for md in range(2):
                cnt = cst.tile([128, N], BF16, tag=f"cn{md}")
                cn.append(cnt)
            mb = cst.tile([128, N], F32, tag="mb")

            pjs = [None, None]

            def psum_exit(dst, srcp, w_):
                if CFG["exit_eng"] == "vector":
                    nc.vector.tensor_copy(dst, srcp)
                    charge("vector", w_ * 1.0417 + 125)
                else:
                    nc.scalar.copy(dst, srcp)
                    charge("scalar", w_ * 0.8333 + 370)

            def qside_proj(ci):
                c0, c1 = CHUNKS[ci]
                w_ = c1 - c0
                pj = []
                for md in range(2):
                    p_ = ps.tile([128, 512], F32, tag="ps")
                    for k in range(8):
                        nc.tensor.matmul(
                            p_[:, :w_], wslice(k, md), q_sb[k][:, c0:c1],
                            start=(k == 0), stop=(k == 7),
                        )
                    pj.append(p_)
                pjs[ci] = pj

            def qside_ln(ci):
                c0, c1 = CHUNKS[ci]
                w_ = c1 - c0
                pj = pjs[ci]
                projb, sq = [], []
                for md in range(2):
                    pb = wrk.tile([128, 512], F32, tag=f"pb{md}")
                    if simple:
                        psum_exit(pb[:, :w_], pj[md][:, :w_], w_)
                    else:
                        nc.vector.tensor_scalar_add(
                            pb[:, :w_], pj[md][:, :w_], bp_sb[:, md : md + 1]
                        )
                        charge("vector", w_ * 1.0417 + 125)
                    projb.append(pb)
                    s = wrk.tile([128, 512], F32, tag=f"sq{md}")
                    nc.gpsimd.tensor_mul(s[:, :w_], pb[:, :w_], pb[:, :w_])
                    charge("gpsimd", w_ * 0.8333)
                    sq.append(s)

                smq = ps.tile([128, 512], F32, tag="ps")
                for md in range(2):
                    nc.tensor.matmul(
                        smq[:, :w_], cq128[:].bitcast(F32R),
                        projb[md][:, :w_].bitcast(F32R),
                        start=(md == 0), stop=(md == 1),
                    )
                sqq = ps.tile([128, 512], F32, tag="ps")
                for md in range(2):
                    nc.tensor.matmul(
                        sqq[:, :w_], cq128[:].bitcast(F32R),
                        sq[md][:, :w_].bitcast(F32R),
                        start=(md == 0), stop=(md == 1),
                    )

                psum_exit(mb[:, c0:c1], smq[:, :w_], w_)
                # cs early: only the final *rstd multiply sits behind rstd
                css = []
                for md in range(2):
                    cs = wrk.tile([128, 512], F32, tag=f"cs{md}")
                    nc.gpsimd.tensor_sub(cs[:, :w_], projb[md][:, :w_], mb[:, c0:c1])
                    charge("gpsimd", w_ * 0.8333)
                    css.append(cs)
                mb2 = wrk.tile([128, 512], F32, tag="mb2")
                nc.gpsimd.tensor_mul(mb2[:, :w_], mb[:, c0:c1], mb[:, c0:c1])
                charge("gpsimd", w_ * 0.8333)
                varq = wrk.tile([128, 512], F32, tag="varq")
                nc.vector.tensor_sub(varq[:, :w_], sqq[:, :w_], mb2[:, :w_])
                charge("vector", w_ * 1.0417 + 125)
                sdq = wrk.tile([128, 512], F32, tag="sdq")
                nc.scalar.activation(sdq[:, :w_], varq[:, :w_], SQRT, bias=eps128[:])
                charge("scalar", w_ * 0.8333 + 370)
                rstd = wrk.tile([128, 512], F32, tag="rstd")
                nc.vector.reciprocal(rstd[:, :w_], sdq[:, :w_])
                charge("vector", w_ * 1.0417 + 60)

                for md in range(2):
                    if simple:
                        nc.gpsimd.tensor_mul(cn[md][:, c0:c1], css[md][:, :w_], rstd[:, :w_])
                        charge("gpsimd", w_ * 0.8333)
                    else:
                        c2_ = wrk.tile([128, 512], F32, tag=f"c2_{md}")
                        nc.gpsimd.tensor_mul(c2_[:, :w_], css[md][:, :w_], rstd[:, :w_])
                        charge("gpsimd", w_ * 0.8333)
                        nc.vector.tensor_scalar(
                            cn[md][:, c0:c1], c2_[:, :w_],
                            g1_sb[:, md : md + 1], b1_sb[:, md : md + 1],
                            op0=MULT, op1=ADD,
                        )
                        charge("vector", w_ * 1.0417 + 60)

            POOL_E = {2, 5, 7}  # e % 8 in POOL_E -> Pool owns both products
            out_rr = [0]

            def emit_tile(e, c0, c1, force_eng=None, force_out=None):
                """products + staging + out DMA for tile column range [c0,c1)."""
                w_ = c1 - c0
                if w_ == N:
                    st = stg.tile([128, 2 * N], BF16, tag="st")
                else:
                    st = stc.tile([128, 2 * 512], BF16, tag="stc")
                sw = st.shape[1] // 2
                meng = force_eng or ("gpsimd" if (e % CFG["pool_mod"][0]) == CFG["pool_mod"][1] else "vector")
                for md in range(2):
                    charge(meng, mul_costs(w_)[meng])
                    getattr(nc, meng).tensor_mul(
                        st[:, sw * md : sw * md + w_],
                        cn[md][:, c0:c1],
                        xbes[e][:, c0:c1],
                    )
                srcv = st[:].rearrange("p (md n) -> p md n", md=2)
                if w_ == N and e >= C2 - CFG["split_last"]:
                    for hi, heng in enumerate(["sync", "scalar"]):
                        h0, h1 = hi * 512, hi * 512 + 512
                        charge(heng, dma_cost(2048))
                        getattr(nc, heng).dma_start(
                            outv[:, :, e, c0 + h0 : c0 + h1],
                            srcv[:, :, h0:h1],
                        )
                    return
                dst = outv[:, :, e, c0:c1]
                src = srcv[:, :, :w_]
                if force_out is not None:
                    eng = force_out
                else:
                    pat = CFG["out_pat"]
                    eng = pat[out_rr[0] % len(pat)]
                    out_rr[0] += 1
                charge(eng, dma_cost(2 * w_ * 2))
                getattr(nc, eng).dma_start(dst, src)

            if CFG["proj_order"] == "bb":
                qside_proj(0)
                qside_proj(1)
                qside_ln(0)
                qside_ln(1)
            else:
                qside_proj(0)
                qside_ln(0)
                qside_proj(1)
                qside_ln(1)
            es = CFG["esplit"]
            ho = CFG["head_out"]
            for e in range(es):
                emit_xbe(e, CFG["pf_eng"][e % len(CFG["pf_eng"])])
                emit_tile(e, 0, 512, force_eng="vector", force_out=ho[e % len(ho)])
            for e in range(es):
                emit_tile(e, 512, N)
            tl = CFG["tail_n"]
            for e in range(es, C2):
                emit_xbe(e)
                emit_xbe(min(e + 6, C2 - 1))
                fo = ["sync", "scalar", "gpsimd"][e % 3] if e >= C2 - tl else None
                emit_tile(e, 0, N, force_out=fo)

    nc.compile()
    return nc


def _host_inputs(q, x, Wp, bp, g1, b1, g2, b2):
    """Build the 8 per-core input maps."""
    import ml_dtypes

    qf = np.asarray(q, dtype=np.float32).reshape(B, C1, N)
    qfb = qf.astype(ml_dtypes.bfloat16)
    xf = np.ascontiguousarray(np.asarray(x, dtype=np.float32).reshape(B, C2, N))
    wT = np.ascontiguousarray(np.asarray(Wp, dtype=np.float32).T).astype(
        ml_dtypes.bfloat16
    )
    bpc = np.ascontiguousarray(np.asarray(bp, dtype=np.float32).reshape(2, 128).T)
    g1c = np.ascontiguousarray(np.asarray(g1, dtype=np.float32).reshape(2, 128).T)
    b1c = np.ascontiguousarray(np.asarray(b1, dtype=np.float32).reshape(2, 128).T)
    g2r = np.ascontiguousarray(np.tile(np.asarray(g2, dtype=np.float32), 4)[:, None])
    b2r = np.ascontiguousarray(np.tile(np.asarray(b2, dtype=np.float32), 4)[:, None])
    in_maps = []
    for b in range(B):
        in_maps.append(
            {
                "qb": np.ascontiguousarray(qfb[b]),
                "wT": wT,
                "x": xf[b],
                "bpc": bpc,
                "g1c": g1c,
                "b1c": b1c,
                "g2r": g2r,
                "b2r": b2r,
            }
        )
    return in_maps


def _is_simple(bp, g1, b1, g2, b2):
    return (
        np.allclose(np.asarray(bp), 0)
        and np.allclose(np.asarray(g1), 1)
        and np.allclose(np.asarray(b1), 0)
        and np.allclose(np.asarray(g2), 1)
        and np.allclose(np.asarray(b2), 0)
    )


def _run(in_maps, simple=True, trace=False):
    from concourse.bass_utils import run_bass_kernel_spmd

    key = f"nc{int(simple)}"
    if key not in _CACHE:
        _CACHE[key] = _build_nc(simple)
    nc = _CACHE[key]
    res = run_bass_kernel_spmd(nc, in_maps, core_ids=list(range(B)), trace=trace)
    return res


def kernel(q, x, Wp, bp, g1, b1, g2, b2):
    simple = _is_simple(bp, g1, b1, g2, b2)
    _CACHE["simple"] = simple
    in_maps = _host_inputs(q, x, Wp, bp, g1, b1, g2, b2)
    res = _run(in_maps, simple=simple, trace=False)
    out = np.stack(
        [
            np.asarray(res.results[b]["out"]).astype(np.float32).reshape(CD, H, W)
            for b in range(B)
        ]
    )
    _CACHE["last_res"] = res
    return out


# revision 35
# speedup vs baseline: 1.0104x; 1.0104x over previous
"""Trainium2 Bass kernel for nn_HadaMard: fused proj + 2xLayerNorm + outer product.

Reference computation (per batch b, one NeuronCore per batch):
  qf = q[b].reshape(C1, N)           # [1024, 1024]
  proj = Wp @ qf (+ bp)              # [256, 1024]
  qn = LN_over_d(proj) * g1 + b1     # LN over the 256-channel dim
  xn = LN_over_e(x[b]) * g2 + b2     # LN over the 32-channel dim
  out[d*32+e, n] = qn[d, n] * xn[e, n]   # [8192, 1024]

Layout/strategy:
  - Output is transferred in bf16 (rel-err ~6e-3 << 2e-2 gate) and upcast on
    host: halves the dominant HBM write traffic.
  - Outer-product tiles are e-major: tile (md, e) holds out rows
    (128*md+p)*32+e for p in [0,128). The qn factor is the bf16 qn tile
    itself (no broadcast); the xn factor is one row broadcast to all 128
    partitions.
  - Row broadcasts go through a DRAM scratch roundtrip: A = xn is written
    once (ready early, x-side only), then each xbe tile is a stride-0
    partition-broadcast DMA read. These land on the DMA queues
    (sync/scalar/gpsimd) during the otherwise-idle window while the q-side
    LN chain runs, instead of loading the busy compute engines.
  - Elementwise products run on DVE (bf16 2x mode) and Pool, DMAs on
    sync/scalar/gpsimd, assigned by a static least-loaded balancer.
  - The q side is processed in two 512-column chunks (PSUM bank granularity);
    early e's run per-chunk products to start output DMA sooner, later e's
    run full-width products.
"""

import numpy as np

_CACHE = {}

B, C1, H, W = 8, 1024, 32, 32
C2 = 32
Cp = 256
N = H * W  # 1024
CD = Cp * C2  # 8192
EPS = 1e-5

CFG = {
    "esplit": 8,        # e < esplit: per-chunk products; else full-width
    "nwu": 2,           # PE warm-up matmuls
    "proj_order": "il", # "bb" = both proj chunks first; "il" = proj/ln interleaved
    "pool_mod": (3, 1), # Pool owns tiles with e % m == r
    "out_pat": ["sync", "scalar", "gpsimd", "sync", "scalar", "sync", "scalar"],
    "xbe_pat": ["gpsimd", "sync", "gpsimd", "scalar"],
    "q_eng": ["sync", "sync", "sync", "scalar", "scalar", "scalar", "gpsimd", "gpsimd"],
    "prefetch": 10,
    "head_out": ["sync"],
    "pf_eng": ["sync"],
    "tail_n": 0,
    "exit_eng": "scalar",
    "split_last": 0,
}


def _build_nc(simple):
    import concourse.bacc as bacc
    import concourse.bass as bass
    import concourse.mybir as mybir
    import concourse.tile as tile

    F32 = mybir.dt.float32
    F32R = mybir.dt.float32r
    BF16 = mybir.dt.bfloat16
    SQRT = mybir.ActivationFunctionType.Sqrt
    COPY = mybir.ActivationFunctionType.Copy
    MULT = mybir.AluOpType.mult
    ADD = mybir.AluOpType.add

    nc = bacc.Bacc(None, target_bir_lowering=False)

    q_d = nc.dram_tensor("qb", [C1, N], BF16, kind="ExternalInput")
    w_d = nc.dram_tensor("wT", [C1, Cp], BF16, kind="ExternalInput")
    x_d = nc.dram_tensor("x", [C2, N], BF16, kind="ExternalInput")
    bp_d = nc.dram_tensor("bpc", [128, 2], F32, kind="ExternalInput")
    g1_d = nc.dram_tensor("g1c", [128, 2], F32, kind="ExternalInput")
    b1_d = nc.dram_tensor("b1c", [128, 2], F32, kind="ExternalInput")
    g2_d = nc.dram_tensor("g2r", [128, 1], F32, kind="ExternalInput")
    b2_d = nc.dram_tensor("b2r", [128, 1], F32, kind="ExternalInput")
    abuf_d = nc.dram_tensor("abuf", [128, 256], BF16, kind="Internal")  # packed: row 32a+e = A[e, 256a:]
    out_d = nc.dram_tensor("out", [CD, N], BF16, kind="ExternalOutput")

    # out view: row (md*128+p)*32+e  ->  [p, md, e, n]
    outv = out_d.rearrange("(md p e) n -> p md e n", md=2, p=128, e=C2)

    # ---- static least-loaded balancer (model-cost ns) ----
    clk = {"sync": 0.0, "scalar": 0.0, "gpsimd": 0.0, "vector": 0.0}

    def pick(cands, costs):
        e = min(cands, key=lambda x: clk[x])
        clk[e] += costs[e] if isinstance(costs, dict) else costs
        return e

    def charge(e, cost):
        clk[e] += cost

    DMA_ENGS = ["sync", "scalar", "gpsimd"]

    def dma_cost(bytes_per_part):
        return max(bytes_per_part * 0.3855, 500.0)

    def mul_costs(w):
        return {"vector": w * 1.0417 * 0.5 + 60.0, "gpsimd": w * 0.8333}

    CHUNKS = [(0, 512), (512, 1024)]

    with tile.TileContext(nc) as tc:
        with (
            tc.tile_pool(name="cst", bufs=1) as cst,
            tc.tile_pool(name="big", bufs=1) as big,
            tc.tile_pool(name="xbe", bufs=1) as xbp,
            tc.tile_pool(name="stg", bufs=7) as stg,
            tc.tile_pool(name="stc", bufs=4) as stc,
            tc.tile_pool(name="wrk", bufs=2) as wrk,
            tc.tile_pool(name="ps", bufs=5, space=bass.MemorySpace.PSUM) as ps,
            tc.tile_pool(name="wups", bufs=1, space=bass.MemorySpace.PSUM) as wups,
            tc.tile_pool(name="ps32", bufs=2, space=bass.MemorySpace.PSUM) as ps32,
        ):
            # ---- constants / warmup (t=0, no input deps) ----
            wu_l = cst.tile([128, 128], BF16, tag="wul")
            nc.vector.memset(wu_l[:], 0.5)
            wu_r = cst.tile([128, 256], BF16, tag="wur")
            nc.vector.memset(wu_r[:], 0.5)
            cq128 = cst.tile([128, 128], BF16, tag="cq128")
            nc.vector.memset(cq128[:], 1.0 / Cp)
            cjx = cst.tile([128, 128], BF16, tag="cjx")
            nc.vector.memset(cjx[:], 0.0)
            for a_ in range(4):
                nc.vector.memset(cjx[32 * a_ : 32 * (a_ + 1), 32 * a_ : 32 * (a_ + 1)], 1.0 / C2)
            eps32 = cst.tile([C2, 1], F32, tag="eps32")
            nc.vector.memset(eps32[:], EPS)
            eps128 = cst.tile([128, 1], F32, tag="eps128")
            nc.vector.memset(eps128[:], EPS)
            # preload the activation table early (ACT, off critical path)
            atl = cst.tile([C2, 1], F32, tag="atl")
            nc.scalar.copy(atl[:], eps32[:])
            nc.scalar.activation(atl[:], eps32[:], SQRT, bias=eps32[:])

            wu_ps = wups.tile([128, 256], F32, tag="wups")
            for i in range(CFG["nwu"]):
                nc.tensor.matmul(wu_ps[:], wu_l[:], wu_r[:], start=True, stop=True)

            def fillers(n):
                for _ in range(n):
                    nc.tensor.matmul(wu_ps[:], wu_l[:], wu_r[:], start=True, stop=True)

            # ---- input loads: q/w spread across all queues for fast proj
            # start; x packed (x4[32a+e, n'] = x[e, 256a+n']) on gpsimd ----
            w_sb = []
            for j in range(2):
                wt = big.tile([128, 4 * Cp], BF16, tag=f"w{j}")
                w_sb.append(wt)
                src = w_d[512 * j : 512 * (j + 1), :].rearrange("(c p) d -> p c d", c=4)
                dst = wt[:].rearrange("p (c d) -> p c d", c=4)
                eng = ["sync", "scalar"][j]
                charge(eng, dma_cost(2048))
                getattr(nc, eng).dma_start(dst, src)
            x_sb = cst.tile([128, 256], BF16, tag="xs")
            charge("gpsimd", dma_cost(512))
            nc.gpsimd.dma_start(
                x_sb[:], x_d.rearrange("e (a n) -> a e n", a=4)
            )
            q_sb = []
            q_eng = CFG["q_eng"]
            for k in range(8):
                qt = big.tile([128, N], BF16, tag=f"q{k}")
                q_sb.append(qt)
                charge(q_eng[k], dma_cost(2048))
                getattr(nc, q_eng[k]).dma_start(qt[:], q_d[128 * k : 128 * (k + 1), :])

            def wslice(k, md):
                j, c = divmod(k, 4)
                return w_sb[j][:, 256 * c + 128 * md : 256 * c + 128 * (md + 1)]

            def cload(dram, shape, tag):
                t = cst.tile(shape, F32, tag=tag)
                eng = pick(DMA_ENGS, dma_cost(shape[1] * 4))
                getattr(nc, eng).dma_start(t[:], dram[:])
                return t

            if not simple:
                bp_sb = cload(bp_d, [128, 2], "bp")
                g1_sb = cload(g1_d, [128, 2], "g1")
                b1_sb = cload(b1_d, [128, 2], "b1")
                g2_sb = cload(g2_d, [128, 1], "g2")
                b2_sb = cload(b2_d, [128, 1], "b2")

            # ---- x-side LN (packed [128,256]; independent of q) ----
            xsq = cst.tile([128, 256], BF16, tag="xsq")
            nc.gpsimd.tensor_mul(xsq[:], x_sb[:], x_sb[:])
            charge("gpsimd", 256 * 0.8333)
            smx = ps32.tile([128, 256], F32, tag="s32")
            nc.tensor.matmul(smx[:], cjx[:], x_sb[:], start=True, stop=True)
            sqx = ps32.tile([128, 256], F32, tag="s32")
            nc.tensor.matmul(sqx[:], cjx[:], xsq[:], start=True, stop=True)
            mx = cst.tile([128, 256], F32, tag="mx")
            nc.scalar.copy(mx[:], smx[:])
            charge("scalar", 256 * 0.8333 + 370)
            mx2 = cst.tile([128, 256], F32, tag="mx2")
            nc.gpsimd.tensor_mul(mx2[:], mx[:], mx[:])
            charge("gpsimd", 256 * 0.8333)
            vx = cst.tile([128, 256], F32, tag="vx")
            nc.vector.tensor_sub(vx[:], sqx[:], mx2[:])
            charge("vector", 256 * 1.0417 + 125)
            sdx = cst.tile([128, 256], F32, tag="sdx")
            nc.scalar.activation(sdx[:], vx[:], SQRT, bias=eps128[:])
            charge("scalar", 256 * 0.8333 + 370)
            rsx = cst.tile([128, 256], F32, tag="rsx")
            nc.vector.reciprocal(rsx[:], sdx[:])
            charge("vector", 256 * 1.0417 + 60)
            xt = cst.tile([128, 256], F32, tag="xt")
            nc.gpsimd.tensor_sub(xt[:], x_sb[:], mx[:])
            charge("gpsimd", 256 * 0.8333)
            a_sb = cst.tile([128, 256], BF16, tag="a")
            if simple:
                nc.gpsimd.tensor_mul(a_sb[:], xt[:], rsx[:])
                charge("gpsimd", 256 * 0.8333)
            else:
                t3 = cst.tile([128, 256], F32, tag="t3")
                nc.gpsimd.tensor_mul(t3[:], xt[:], rsx[:])
                charge("gpsimd", 256 * 0.8333)
                nc.vector.tensor_scalar(
                    a_sb[:], t3[:], g2_sb[:, 0:1], b2_sb[:, 0:1], op0=MULT, op1=ADD
                )
                charge("vector", 256 * 1.0417 + 60)

            # A -> DRAM scratch (packed), then all 32 broadcast reads (fill
            # the DMA window while the q-side LN chain runs)
            charge("sync", dma_cost(512))
            nc.sync.dma_start(abuf_d[:], a_sb[:])
            abufv = abuf_d.rearrange("(a e) n -> e a n", a=4, e=C2)
            xbes = [None] * C2
            xbe_rr = [0]

            def emit_xbe(e, eng=None):
                if xbes[e] is not None:
                    return
                t = xbp.tile([128, N], BF16, tag=f"xbe{e}")
                src = abufv[e : e + 1, :, :].partition_broadcast(128)
                if eng is None:
                    pat = CFG["xbe_pat"]
                    eng = pat[xbe_rr[0] % len(pat)]
                    xbe_rr[0] += 1
                charge(eng, dma_cost(N * 2))
                getattr(nc, eng).dma_start(
                    t[:].rearrange("p (a n) -> p a n", a=4), src
                )
                xbes[e] = t

            pfe = CFG["pf_eng"]
            for e in range(CFG["prefetch"]):
                emit_xbe(e, pfe[e % len(pfe)])

            # ---- q-side: proj + LN -> cn (bf16, rstd folded in) ----
            cn = []
            for md in range(2):
                cnt = cst.tile([128, N], BF16, tag=f"cn{md}")
                cn.append(cnt)
            mb = cst.tile([128, N], F32, tag="mb")

            pjs = [None, None]

            def psum_exit(dst, srcp, w_):
                if CFG["exit_eng"] == "vector":
                    nc.vector.tensor_copy(dst, srcp)
                    charge("vector", w_ * 1.0417 + 125)
                else:
                    nc.scalar.copy(dst, srcp)
                    charge("scalar", w_ * 0.8333 + 370)

            def qside_proj(ci):
                c0, c1 = CHUNKS[ci]
                w_ = c1 - c0
                pj = []
                for md in range(2):
                    p_ = ps.tile([128, 512], F32, tag="ps")
                    for k in range(8):
                        nc.tensor.matmul(
                            p_[:, :w_], wslice(k, md), q_sb[k][:, c0:c1],
                            start=(k == 0), stop=(k == 7),
                        )
                    pj.append(p_)
                pjs[ci] = pj

            def qside_ln(ci):
                c0, c1 = CHUNKS[ci]
                w_ = c1 - c0
                pj = pjs[ci]
                projb, sq, projb_bf = [], [], []
                for md in range(2):
                    pb = wrk.tile([128, 512], F32, tag=f"pb{md}")
                    if simple:
                        psum_exit(pb[:, :w_], pj[md][:, :w_], w_)
                    else:
                        nc.vector.tensor_scalar_add(
                            pb[:, :w_], pj[md][:, :w_], bp_sb[:, md : md + 1]
                        )
                        charge("vector", w_ * 1.0417 + 125)
                    projb.append(pb)
                    s = wrk.tile([128, 512], BF16, tag=f"sq{md}")
                    nc.gpsimd.tensor_mul(s[:, :w_], pb[:, :w_], pb[:, :w_])
                    charge("gpsimd", w_ * 0.8333)
                    sq.append(s)
                    pbb = wrk.tile([128, 512], BF16, tag=f"pbb{md}")
                    nc.gpsimd.tensor_copy(pbb[:, :w_], pb[:, :w_])
                    charge("gpsimd", w_ * 0.8333)
                    projb_bf.append(pbb)

                smq = ps.tile([128, 512], F32, tag="ps")
                for md in range(2):
                    nc.tensor.matmul(
                        smq[:, :w_], cq128[:], projb_bf[md][:, :w_],
                        start=(md == 0), stop=(md == 1),
                    )
                sqq = ps.tile([128, 512], F32, tag="ps")
                for md in range(2):
                    nc.tensor.matmul(
                        sqq[:, :w_], cq128[:], sq[md][:, :w_],
                        start=(md == 0), stop=(md == 1),
                    )

                psum_exit(mb[:, c0:c1], smq[:, :w_], w_)
                # cs early: only the final *rstd multiply sits behind rstd
                css = []
                for md in range(2):
                    cs = wrk.tile([128, 512], F32, tag=f"cs{md}")
                    nc.gpsimd.tensor_sub(cs[:, :w_], projb[md][:, :w_], mb[:, c0:c1])
                    charge("gpsimd", w_ * 0.8333)
                    css.append(cs)
                mb2 = wrk.tile([128, 512], F32, tag="mb2")
                nc.gpsimd.tensor_mul(mb2[:, :w_], mb[:, c0:c1], mb[:, c0:c1])
                charge("gpsimd", w_ * 0.8333)
                varq = wrk.tile([128, 512], F32, tag="varq")
                nc.vector.tensor_sub(varq[:, :w_], sqq[:, :w_], mb2[:, :w_])
                charge("vector", w_ * 1.0417 + 125)
                sdq = wrk.tile([128, 512], F32, tag="sdq")
                nc.scalar.activation(sdq[:, :w_], varq[:, :w_], SQRT, bias=eps128[:])
                charge("scalar", w_ * 0.8333 + 370)
                rstd = wrk.tile([128, 512], F32, tag="rstd")
                nc.vector.reciprocal(rstd[:, :w_], sdq[:, :w_])
                charge("vector", w_ * 1.0417 + 60)

                for md in range(2):
                    if simple:
                        nc.gpsimd.tensor_mul(cn[md][:, c0:c1], css[md][:, :w_], rstd[:, :w_])
                        charge("gpsimd", w_ * 0.8333)
                    else:
                        c2_ = wrk.tile([128, 512], F32, tag=f"c2_{md}")
                        nc.gpsimd.tensor_mul(c2_[:, :w_], css[md][:, :w_], rstd[:, :w_])
                        charge("gpsimd", w_ * 0.8333)
                        nc.vector.tensor_scalar(
                            cn[md][:, c0:c1], c2_[:, :w_],
                            g1_sb[:, md : md + 1], b1_sb[:, md : md + 1],
                            op0=MULT, op1=ADD,
                        )
                        charge("vector", w_ * 1.0417 + 60)

            POOL_E = {2, 5, 7}  # e % 8 in POOL_E -> Pool owns both products
            out_rr = [0]

            def emit_tile(e, c0, c1, force_eng=None, force_out=None):
                """products + staging + out DMA for tile column range [c0,c1)."""
                w_ = c1 - c0
                if w_ == N:
                    st = stg.tile([128, 2 * N], BF16, tag="st")
                else:
                    st = stc.tile([128, 2 * 512], BF16, tag="stc")
                sw = st.shape[1] // 2
                meng = force_eng or ("gpsimd" if (e % CFG["pool_mod"][0]) == CFG["pool_mod"][1] else "vector")
                for md in range(2):
                    charge(meng, mul_costs(w_)[meng])
                    getattr(nc, meng).tensor_mul(
                        st[:, sw * md : sw * md + w_],
                        cn[md][:, c0:c1],
                        xbes[e][:, c0:c1],
                    )
                srcv = st[:].rearrange("p (md n) -> p md n", md=2)
                if w_ == N and e >= C2 - CFG["split_last"]:
                    for hi, heng in enumerate(["sync", "scalar"]):
                        h0, h1 = hi * 512, hi * 512 + 512
                        charge(heng, dma_cost(2048))
                        getattr(nc, heng).dma_start(
                            outv[:, :, e, c0 + h0 : c0 + h1],
                            srcv[:, :, h0:h1],
                        )
                    return
                dst = outv[:, :, e, c0:c1]
                src = srcv[:, :, :w_]
                if force_out is not None:
                    eng = force_out
                else:
                    pat = CFG["out_pat"]
                    eng = pat[out_rr[0] % len(pat)]
                    out_rr[0] += 1
                charge(eng, dma_cost(2 * w_ * 2))
                getattr(nc, eng).dma_start(dst, src)

            if CFG["proj_order"] == "bb":
                qside_proj(0)
                qside_proj(1)
                qside_ln(0)
                qside_ln(1)
            else:
                qside_proj(0)
                qside_ln(0)
                qside_proj(1)
                qside_ln(1)
            es = CFG["esplit"]
            ho = CFG["head_out"]
            for e in range(es):
                emit_xbe(e, CFG["pf_eng"][e % len(CFG["pf_eng"])])
                emit_tile(e, 0, 512, force_eng="vector", force_out=ho[e % len(ho)])
            for e in range(es):
                emit_tile(e, 512, N)
            tl = CFG["tail_n"]
            for e in range(es, C2):
                emit_xbe(e)
                emit_xbe(min(e + 6, C2 - 1))
                fo = ["sync", "scalar", "gpsimd"][e % 3] if e >= C2 - tl else None
                emit_tile(e, 0, N, force_out=fo)

    nc.compile()
    return nc


def _host_inputs(q, x, Wp, bp, g1, b1, g2, b2):
    """Build the 8 per-core input maps."""
    import ml_dtypes

    qf = np.asarray(q, dtype=np.float32).reshape(B, C1, N)
    qfb = qf.astype(ml_dtypes.bfloat16)
    xf = np.ascontiguousarray(np.asarray(x, dtype=np.float32).reshape(B, C2, N))
    wT = np.ascontiguousarray(np.asarray(Wp, dtype=np.float32).T).astype(
        ml_dtypes.bfloat16
    )
    bpc = np.ascontiguousarray(np.asarray(bp, dtype=np.float32).reshape(2, 128).T)
    g1c = np.ascontiguousarray(np.asarray(g1, dtype=np.float32).reshape(2, 128).T)
    b1c = np.ascontiguousarray(np.asarray(b1, dtype=np.float32).reshape(2, 128).T)
    g2r = np.ascontiguousarray(np.tile(np.asarray(g2, dtype=np.float32), 4)[:, None])
    b2r = np.ascontiguousarray(np.tile(np.asarray(b2, dtype=np.float32), 4)[:, None])
    in_maps = []
    for b in range(B):
        in_maps.append(
            {
                "qb": np.ascontiguousarray(qfb[b]),
                "wT": wT,
                "x": xf[b],
                "bpc": bpc,
                "g1c": g1c,
                "b1c": b1c,
                "g2r": g2r,
                "b2r": b2r,
            }
        )
    return in_maps


def _is_simple(bp, g1, b1, g2, b2):
    return (
        np.allclose(np.asarray(bp), 0)
        and np.allclose(np.asarray(g1), 1)
        and np.allclose(np.asarray(b1), 0)
        and np.allclose(np.asarray(g2), 1)
        and np.allclose(np.asarray(b2), 0)
    )


def _run(in_maps, simple=True, trace=False):
    from concourse.bass_utils import run_bass_kernel_spmd

    key = f"nc{int(simple)}"
    if key not in _CACHE:
        _CACHE[key] = _build_nc(simple)
    nc = _CACHE[key]
    res = run_bass_kernel_spmd(nc, in_maps, core_ids=list(range(B)), trace=trace)
    return res


def kernel(q, x, Wp, bp, g1, b1, g2, b2):
    simple = _is_simple(bp, g1, b1, g2, b2)
    _CACHE["simple"] = simple
    in_maps = _host_inputs(q, x, Wp, bp, g1, b1, g2, b2)
    res = _run(in_maps, simple=simple, trace=False)
    out = np.stack(
        [
            np.asarray(res.results[b]["out"]).astype(np.float32).reshape(CD, H, W)
            for b in range(B)
        ]
    )
    _CACHE["last_res"] = res
    return out
